# revision 3
# baseline (speedup 1.0000x reference)
"""CTC loss forward on 8 TRN2 NeuronCores, data-parallel over batch.

Problem: log_probs (512, 32, 8000) f32, targets (32, 40) i32,
target_lengths (32,) i32 -> per-sample loss (32,) f32
(input_lengths is ignored, matching the reference).

Strategy per core (4 samples):
 - Gather only the needed log-prob entries: glp[s, t, n] = lp[t, n, et[n, s]]
   (T*4*81 = 166K elements) via one indirect DMA; the 512MB tensor is
   never streamed.
 - Run the T-step DP in linear probability space, states (Se=81) on
   partitions: alpha' = (W1 @ alpha + S2 @ (mask*alpha)) * p_t where the
   two shift matmuls accumulate in PSUM and one fused DVE multiply forms
   both alpha' and mask*alpha' (X = [alpha | am], 8 free columns).
 - Every K=8 steps renormalize by the per-sample sum over states
   s <= 2L (window excludes padding states that run away), clamp, and
   log-accumulate the scales.
 - loss = -(log(alpha[2L] + alpha[2L-1]) + sum(log scales) - T*SHIFT)/L
"""
import sys

for _p in ("/opt/trn_rl_repo",):
    if _p not in sys.path:
        sys.path.append(_p)

import numpy as np
import concourse.bass as bass
import concourse.bacc as bacc
import concourse.mybir as mybir
from concourse import tile
from concourse.bass_utils import run_bass_kernel_spmd

F32 = mybir.dt.float32
I32 = mybir.dt.int32
AF = mybir.ActivationFunctionType
OP = mybir.AluOpType

T_FULL = 512
NL = 4          # samples per core
NC_CORES = 8
C = 8000
S = 40
SE = 2 * S + 1  # 81
K_RENORM = 8
SHIFT = 9.0
E_SHIFT = float(np.float32(np.exp(np.float32(SHIFT))))
CLAMP = 1e26


def _ap(t, off, dims):
    a = t[:]
    return bass.AP(a.tensor, off, [list(d) for d in dims])


def build_nc(T=T_FULL):
    nc = bacc.Bacc("TRN2", target_bir_lowering=False, debug=True)
    lp_ext = nc.declare_dram_parameter("log_probs", [T, NL, C], F32, isOutput=False)
    tg_ext = nc.declare_dram_parameter("targets", [NL, S], I32, isOutput=False)
    tl_ext = nc.declare_dram_parameter("target_lengths", [NL], I32, isOutput=False)
    out_ext = nc.declare_dram_parameter("out", [1, NL], F32, isOutput=True)

    n_ren = len([t for t in range(1, T) if t % K_RENORM == 0 and t != T - 1]) + 1

    with tile.TileContext(nc) as tc:
        with (
            tc.tile_pool(name="cst", bufs=1) as cst,
            tc.tile_pool(name="big", bufs=1) as big,
            tc.tile_pool(name="x", bufs=3) as xpool,
            tc.tile_pool(name="tmp", bufs=2) as tmp,
            tc.tile_pool(name="ps", bufs=2, space=bass.MemorySpace.PSUM) as psp,
            tc.tile_pool(name="ps1", bufs=2, space=bass.MemorySpace.PSUM) as ps1,
        ):
            # ---------- constants built on device ----------
            dmat = cst.tile([128, 128], I32, tag="dmat")
            nc.gpsimd.iota(dmat[:], pattern=[[1, 128]], base=0, channel_multiplier=-1)
            ident = cst.tile([128, 128], F32, tag="ident")
            nc.vector.tensor_scalar(ident[:], dmat[:], 0, None, OP.is_equal)
            # W1^T[c, o] = 1 iff o in {c, c+1}; S2^T[c, o] = 1 iff o == c+2
            w1 = cst.tile([SE, SE], F32, tag="w1")
            tmp_ge = tmp.tile([SE, SE], F32, tag="scr0")
            nc.vector.tensor_scalar(tmp_ge[:], dmat[:SE, :SE], 0, None, OP.is_ge)
            tmp_le = tmp.tile([SE, SE], F32, tag="scr1")
            nc.vector.tensor_scalar(tmp_le[:], dmat[:SE, :SE], 1, None, OP.is_le)
            nc.vector.tensor_mul(w1[:], tmp_ge[:], tmp_le[:])
            s2 = cst.tile([SE, SE], F32, tag="s2")
            nc.vector.tensor_scalar(s2[:], dmat[:SE, :SE], 2, None, OP.is_equal)
            onesl = cst.tile([SE, 1], F32, tag="onesl")
            nc.vector.memset(onesl[:], 1.0)
            onesb = cst.tile([1, SE], F32, tag="onesb")
            nc.vector.memset(onesb[:], 1.0)

            # ---------- small inputs ----------
            tgs = cst.tile([NL, S], I32, tag="tgs")
            nc.sync.dma_start(tgs[:], tg_ext[:])
            tls = cst.tile([NL, 1], I32, tag="tls")
            nc.sync.dma_start(tls[:], _ap(tl_ext, 0, [[1, NL], [1, 1]]))

            # et (NL, SE) f32: blank-expanded targets; odd slots get labels
            et = cst.tile([NL, SE], F32, tag="et")
            nc.vector.memset(et[:], 0.0)
            nc.vector.tensor_copy(_ap(et, 1, [[SE, NL], [2, S]]), tgs[:])
            # mfree (NL, SE): col s' holds mask at dest s'+2, i.e.
            # (et[s'+2] != et[s']) for s' <= SE-3, else 0
            mfree = cst.tile([NL, SE], F32, tag="mfree")
            nc.vector.memset(mfree[:], 0.0)
            nc.vector.tensor_tensor(
                _ap(mfree, 0, [[SE, NL], [1, SE - 2]]),
                _ap(et, 2, [[SE, NL], [1, SE - 2]]),
                _ap(et, 0, [[SE, NL], [1, SE - 2]]),
                OP.not_equal,
            )

            # ---------- transposes to (SE, NL) layout ----------
            etT_ps = ps1.tile([SE, NL], F32, tag="tp")
            nc.tensor.transpose(etT_ps[:], et[:], ident[:NL, :NL])
            etT_i = cst.tile([SE, NL], I32, tag="etTi")
            nc.vector.tensor_copy(etT_i[:], etT_ps[:])
            mT_ps = ps1.tile([SE, NL], F32, tag="tp")
            nc.tensor.transpose(mT_ps[:], mfree[:], ident[:NL, :NL])
            mT = cst.tile([SE, NL], F32, tag="mT")
            nc.vector.tensor_copy(mT[:], mT_ps[:])
            # target lengths -> row (1, NL) f32, then 2L
            tlf = cst.tile([NL, 1], F32, tag="tlf")
            nc.vector.tensor_copy(tlf[:], tls[:])
            tlT_ps = ps1.tile([1, NL], F32, tag="tp1")
            nc.tensor.transpose(tlT_ps[:], tlf[:], ident[:NL, :NL])
            lrow = cst.tile([1, NL], F32, tag="lrow")
            nc.vector.tensor_copy(lrow[:], tlT_ps[:])
            l2row = cst.tile([1, NL], F32, tag="l2row")
            nc.vector.tensor_scalar(l2row[:], lrow[:], 2.0, None, OP.mult)
            # thr (SE, NL) = 2L broadcast down partitions (via PE)
            thr_ps = ps1.tile([SE, NL], F32, tag="tp")
            nc.tensor.matmul(thr_ps[:], onesb[:1, :SE], l2row[:], start=True, stop=True)
            thr = cst.tile([SE, NL], F32, tag="thr")
            nc.vector.tensor_copy(thr[:], thr_ps[:])

            # Wm (SE, 2*NL): 1 iff s <= 2L, duplicated halves
            siota = cst.tile([SE, 1], I32, tag="siota")
            nc.gpsimd.iota(siota[:], pattern=[[0, 1]], base=0, channel_multiplier=1)
            siof = cst.tile([SE, 1], F32, tag="siof")
            nc.vector.tensor_copy(siof[:], siota[:])
            wm = cst.tile([SE, 2 * NL], F32, tag="wm")
            nc.vector.tensor_tensor(
                wm[:],
                _ap(siof, 0, [[1, SE], [0, 2], [0, NL]]),
                _ap(thr, 0, [[NL, SE], [0, 2], [1, NL]]),
                OP.is_le,
            )

            # ---------- gather offsets + indirect gather ----------
            offs = big.tile([SE, T * NL], I32, tag="offs")
            nc.gpsimd.iota(
                _ap(offs, 0, [[T * NL, SE], [NL, T], [1, NL]]),
                pattern=[[NL * C, T], [C, NL]],
                base=0,
                channel_multiplier=0,
            )
            nc.vector.tensor_tensor(
                offs[:],
                offs[:],
                _ap(etT_i, 0, [[NL, SE], [0, T], [1, NL]]),
                OP.add,
            )
            glp = big.tile([SE, T * NL], F32, tag="glp")
            nc.gpsimd.indirect_dma_start(
                glp[:],
                None,
                bass.AP(lp_ext, 0, [[C, T * NL], [1, C]]),
                bass.IndirectOffsetOnAxis(ap=offs[:], axis=1),
            )

            # ---------- P~ (SE, T*8): per t, [p (NL) | p*m' (NL)] ----------
            ptil = big.tile([SE, T * 2 * NL], F32, tag="ptil")
            # p = exp(glp) * e^SHIFT
            nc.scalar.activation(
                _ap(ptil, 0, [[T * 2 * NL, SE], [2 * NL, T], [1, NL]]),
                glp[:],
                AF.Exp,
            )
            nc.vector.tensor_scalar(
                _ap(ptil, 0, [[T * 2 * NL, SE], [2 * NL, T], [1, NL]]),
                _ap(ptil, 0, [[T * 2 * NL, SE], [2 * NL, T], [1, NL]]),
                E_SHIFT,
                None,
                OP.mult,
            )
            nc.vector.tensor_tensor(
                _ap(ptil, NL, [[T * 2 * NL, SE], [2 * NL, T], [1, NL]]),
                _ap(ptil, 0, [[T * 2 * NL, SE], [2 * NL, T], [1, NL]]),
                _ap(mT, 0, [[NL, SE], [0, T], [1, NL]]),
                OP.mult,
            )

            # ---------- scan ----------
            rlog = cst.tile([1, n_ren * NL], F32, tag="rlog")

            x = xpool.tile([SE, 2 * NL], F32, tag="X")
            nc.vector.memset(x[:], 0.0)
            nc.vector.tensor_copy(
                _ap(x, 0, [[2 * NL, 2], [1, NL]]),
                _ap(ptil, 0, [[T * 2 * NL, 2], [1, NL]]),
            )
            nc.vector.tensor_tensor(
                _ap(x, NL, [[2 * NL, SE], [1, NL]]),
                _ap(x, 0, [[2 * NL, SE], [1, NL]]),
                mT[:],
                OP.mult,
            )

            jren = 0
            for t in range(1, T):
                acc = psp.tile([SE, NL], F32, tag="acc")
                nc.tensor.matmul(
                    acc[:], w1[:], _ap(x, 0, [[2 * NL, SE], [1, NL]]),
                    start=True, stop=False,
                )
                nc.tensor.matmul(
                    acc[:], s2[:], _ap(x, NL, [[2 * NL, SE], [1, NL]]),
                    start=False, stop=True,
                )
                xn = xpool.tile([SE, 2 * NL], F32, tag="X")
                nc.vector.tensor_tensor(
                    xn[:],
                    _ap(acc, 0, [[NL, SE], [0, 2], [1, NL]]),
                    _ap(ptil, t * 2 * NL, [[T * 2 * NL, SE], [1, 2 * NL]]),
                    OP.mult,
                )
                x = xn

                if (t % K_RENORM == 0 and t != T - 1) or t == T - 1:
                    aw = xpool.tile([SE, 2 * NL], F32, tag="X")
                    nc.vector.tensor_tensor(aw[:], x[:], wm[:], OP.mult)
                    rs = ps1.tile([1, NL], F32, tag="tp1")
                    nc.tensor.matmul(
                        rs[:], onesl[:], _ap(aw, 0, [[2 * NL, SE], [1, NL]]),
                        start=True, stop=True,
                    )
                    rr = tmp.tile([1, NL], F32, tag="rr")
                    nc.vector.reciprocal(rr[:], rs[:])
                    nc.vector.tensor_copy(
                        _ap(rlog, jren, [[n_ren * NL, 1], [n_ren, NL]]),
                        rs[:],
                    )
                    rb = ps1.tile([SE, NL], F32, tag="tp")
                    nc.tensor.matmul(
                        rb[:], onesb[:1, :SE], rr[:], start=True, stop=True
                    )
                    xs = xpool.tile([SE, 2 * NL], F32, tag="X")
                    nc.vector.tensor_tensor(
                        xs[:],
                        aw[:],
                        _ap(rb, 0, [[NL, SE], [0, 2], [1, NL]]),
                        OP.mult,
                    )
                    xr = xpool.tile([SE, 2 * NL], F32, tag="X")
                    nc.vector.tensor_scalar(xr[:], xs[:], CLAMP, None, OP.min)
                    x = xr
                    jren += 1
            assert jren == n_ren

            # ---------- final extraction ----------
            thrm1 = tmp.tile([SE, NL], F32, tag="thrm1")
            nc.vector.tensor_scalar(thrm1[:], thr[:], 1.0, None, OP.subtract)
            ge = tmp.tile([SE, NL], F32, tag="ge")
            nc.vector.tensor_tensor(
                ge[:], _ap(siof, 0, [[1, SE], [0, NL]]), thrm1[:], OP.is_ge
            )
            le = tmp.tile([SE, NL], F32, tag="le")
            nc.vector.tensor_tensor(
                le[:], _ap(siof, 0, [[1, SE], [0, NL]]), thr[:], OP.is_le
            )
            wsel = tmp.tile([SE, NL], F32, tag="wsel")
            nc.vector.tensor_mul(wsel[:], ge[:], le[:])
            sel = tmp.tile([SE, NL], F32, tag="sel")
            nc.vector.tensor_mul(sel[:], _ap(x, 0, [[2 * NL, SE], [1, NL]]), wsel[:])
            tot = ps1.tile([1, NL], F32, tag="tp1")
            nc.tensor.matmul(tot[:], onesl[:], sel[:], start=True, stop=True)
            ltot = tmp.tile([1, NL], F32, tag="ltot")
            nc.scalar.activation(ltot[:], tot[:], AF.Ln)
            # log of scales, then sum over renorm events (n-major layout)
            lr = tmp.tile([1, n_ren * NL], F32, tag="lr")
            nc.scalar.activation(lr[:], rlog[:], AF.Ln)
            slog = tmp.tile([1, NL], F32, tag="slog")
            nc.vector.tensor_reduce(
                slog[:],
                _ap(lr, 0, [[n_ren * NL, 1], [n_ren, NL], [1, n_ren]]),
                mybir.AxisListType.X,
                OP.add,
            )
            q = tmp.tile([1, NL], F32, tag="q")
            nc.vector.tensor_add(q[:], ltot[:], slog[:])
            # (q - T*SHIFT) * -1 = T*SHIFT - q
            q2 = tmp.tile([1, NL], F32, tag="q2")
            nc.vector.tensor_scalar(q2[:], q[:], float(T) * SHIFT, -1.0, OP.subtract, OP.mult)
            rl = tmp.tile([1, NL], F32, tag="rl")
            nc.vector.reciprocal(rl[:], lrow[:])
            loss = tmp.tile([1, NL], F32, tag="loss")
            nc.vector.tensor_mul(loss[:], q2[:], rl[:])
            nc.sync.dma_start(out_ext[:], loss[:])

    nc.compile()
    return nc


_NC_CACHE = {}


def _get_nc(T=T_FULL):
    if T not in _NC_CACHE:
        _NC_CACHE[T] = build_nc(T)
    return _NC_CACHE[T]


def kernel(log_probs, targets, input_lengths, target_lengths):
    lp = np.ascontiguousarray(np.asarray(log_probs, dtype=np.float32))
    tg = np.ascontiguousarray(np.asarray(targets, dtype=np.int32))
    tl = np.ascontiguousarray(np.asarray(target_lengths, dtype=np.int32))
    T, N, _ = lp.shape
    nc = _get_nc(T)
    in_maps = []
    for i in range(NC_CORES):
        s = slice(i * NL, (i + 1) * NL)
        in_maps.append(
            {
                "log_probs": np.ascontiguousarray(lp[:, s, :]),
                "targets": np.ascontiguousarray(tg[s]),
                "target_lengths": np.ascontiguousarray(tl[s]),
            }
        )
    res = run_bass_kernel_spmd(nc, in_maps, core_ids=list(range(NC_CORES)))
    out = np.concatenate([res.results[i]["out"].reshape(NL) for i in range(NC_CORES)])
    return out.astype(np.float32)


# revision 5
# speedup vs baseline: 1.1177x; 1.1177x over previous
"""CTC loss forward on 8 TRN2 NeuronCores, data-parallel over batch.

Problem: log_probs (512, 32, 8000) f32, targets (32, 40) i32,
target_lengths (32,) i32 -> per-sample loss (32,) f32
(input_lengths is ignored, matching the reference).

Strategy per core (4 samples):
 - Gather only the needed log-prob entries: glp[s, t, n] = lp[t, n, et[n, s]]
   (T*4*81 = 166K elements) via one indirect DMA; the 512MB tensor is
   never streamed.
 - Run the T-step DP in linear probability space with an augmented state
   on partitions: rows 0..80 = alpha over the 81 CTC states, rows
   81..119 = the 39 masked skip terms am[j] = alpha[2j+1]*mask[2j+3].
   One constant 120x120 matmul performs all shifts AND regenerates the
   duplicated skip rows; one FD=4 DVE multiply by the precomputed
   per-step probability page completes the step:
       X' = (W2 @ X) * P2[:, t]
 - Every K=8 steps renormalize by the per-sample sum of alpha over
   states s <= 2L (window excludes padding states that run away),
   clamp, and log-accumulate the scales.
 - loss = -(log(alpha[2L] + alpha[2L-1]) + sum(log scales) - T*SHIFT)/L
"""
import sys

for _p in ("/opt/trn_rl_repo",):
    if _p not in sys.path:
        sys.path.append(_p)

import numpy as np
import concourse.bass as bass
import concourse.bacc as bacc
import concourse.mybir as mybir
from concourse import tile
from concourse.bass_utils import run_bass_kernel_spmd

F32 = mybir.dt.float32
I32 = mybir.dt.int32
AF = mybir.ActivationFunctionType
OP = mybir.AluOpType

T_FULL = 512
NL = 4          # samples per core
NC_CORES = 8
C = 8000
S = 40
SE = 2 * S + 1  # 81
NJ = 39         # skip rows: odd states 1,3,..,77
NP = SE + NJ    # 120 partitions of augmented state
K_RENORM = 8
SHIFT = 9.0
E_SHIFT = float(np.float32(np.exp(np.float32(SHIFT))))
CLAMP = 1e26


def _ap(t, off, dims):
    a = t[:]
    return bass.AP(a.tensor, off, [list(d) for d in dims])


def build_nc(T=T_FULL):
    nc = bacc.Bacc("TRN2", target_bir_lowering=False, debug=True)
    lp_ext = nc.declare_dram_parameter("log_probs", [T, NL, C], F32, isOutput=False)
    tg_ext = nc.declare_dram_parameter("targets", [NL, S], I32, isOutput=False)
    tl_ext = nc.declare_dram_parameter("target_lengths", [NL], I32, isOutput=False)
    out_ext = nc.declare_dram_parameter("out", [1, NL], F32, isOutput=True)

    n_ren = len([t for t in range(1, T) if t % K_RENORM == 0 and t != T - 1]) + 1

    with tile.TileContext(nc) as tc:
        with (
            tc.tile_pool(name="cst", bufs=1) as cst,
            tc.tile_pool(name="big", bufs=1) as big,
            tc.tile_pool(name="x", bufs=3) as xpool,
            tc.tile_pool(name="tmp", bufs=2) as tmp,
            tc.tile_pool(name="ps", bufs=2, space=bass.MemorySpace.PSUM) as psp,
            tc.tile_pool(name="ps1", bufs=2, space=bass.MemorySpace.PSUM) as ps1,
        ):
            # ---------- constants built on device ----------
            dmat = cst.tile([128, 128], I32, tag="dmat")
            nc.gpsimd.iota(dmat[:], pattern=[[1, 128]], base=0, channel_multiplier=-1)
            ident = cst.tile([128, 128], F32, tag="ident")
            nc.vector.tensor_scalar(ident[:], dmat[:], 0, None, OP.is_equal)

            # W2 lhsT (NP, NP): lhsT[c, o] = W2[o, c]
            w2 = cst.tile([NP, NP], F32, tag="w2")
            nc.vector.memset(w2[:], 0.0)
            # [0:81, 0:81]: 1 iff o - c in {0, 1}
            ge0 = tmp.tile([SE, SE], F32, tag="scr0")
            nc.vector.tensor_scalar(ge0[:], dmat[:SE, :SE], 0, None, OP.is_ge)
            le1 = tmp.tile([SE, SE], F32, tag="scr1")
            nc.vector.tensor_scalar(le1[:], dmat[:SE, :SE], 1, None, OP.is_le)
            nc.vector.tensor_mul(_ap(w2, 0, [[NP, SE], [1, SE]]), ge0[:], le1[:])
            # [0:81, 81:120]: lhsT[c, 81+j] = 1 iff c - 2j in {0, 1}
            i2 = cst.tile([SE, NJ], I32, tag="i2")
            nc.gpsimd.iota(i2[:], pattern=[[-2, NJ]], base=0, channel_multiplier=1)
            gA = tmp.tile([SE, NJ], F32, tag="gA")
            nc.vector.tensor_scalar(gA[:], i2[:], 0, None, OP.is_ge)
            gB = tmp.tile([SE, NJ], F32, tag="gB")
            nc.vector.tensor_scalar(gB[:], i2[:], 1, None, OP.is_le)
            nc.vector.tensor_mul(_ap(w2, SE, [[NP, SE], [1, NJ]]), gA[:], gB[:])
            # rows 81:120 built at base partitions, then DMA'd into place
            scrI = cst.tile([NJ, NP], I32, tag="scrI")
            # cols 0:81: 1 iff f - 2j - 3 == 0
            nc.gpsimd.iota(
                _ap(scrI, 0, [[NP, NJ], [1, SE]]),
                pattern=[[1, SE]], base=-3, channel_multiplier=-2,
            )
            # cols 81:120: 1 iff f - j - 1 == 0
            nc.gpsimd.iota(
                _ap(scrI, SE, [[NP, NJ], [1, NJ]]),
                pattern=[[1, NJ]], base=-1, channel_multiplier=-1,
            )
            scrF = cst.tile([NJ, NP], F32, tag="scrF")
            nc.vector.tensor_scalar(scrF[:], scrI[:], 0, None, OP.is_equal)
            nc.sync.dma_start(_ap(w2, SE * NP, [[NP, NJ], [1, NP]]), scrF[:])

            onesl = cst.tile([SE, 1], F32, tag="onesl")
            nc.vector.memset(onesl[:], 1.0)
            onesb = cst.tile([1, NP], F32, tag="onesb")
            nc.vector.memset(onesb[:], 1.0)

            # ---------- small inputs ----------
            tgs = cst.tile([NL, S], I32, tag="tgs")
            nc.sync.dma_start(tgs[:], tg_ext[:])
            tls = cst.tile([NL, 1], I32, tag="tls")
            nc.sync.dma_start(tls[:], _ap(tl_ext, 0, [[1, NL], [1, 1]]))

            # et (NL, SE) f32: blank-expanded targets; odd slots get labels
            et = cst.tile([NL, SE], F32, tag="et")
            nc.vector.memset(et[:], 0.0)
            nc.vector.tensor_copy(_ap(et, 1, [[SE, NL], [2, S]]), tgs[:])
            # mfree (NL, SE): col s' holds mask at dest s'+2 = (et[s'+2] != et[s'])
            mfree = cst.tile([NL, SE], F32, tag="mfree")
            nc.vector.memset(mfree[:], 0.0)
            nc.vector.tensor_tensor(
                _ap(mfree, 0, [[SE, NL], [1, SE - 2]]),
                _ap(et, 2, [[SE, NL], [1, SE - 2]]),
                _ap(et, 0, [[SE, NL], [1, SE - 2]]),
                OP.not_equal,
            )

            # ---------- transposes to states-on-partitions ----------
            etT_ps = ps1.tile([SE, NL], F32, tag="tp")
            nc.tensor.transpose(etT_ps[:], et[:], ident[:NL, :NL])
            etT_i = cst.tile([SE, NL], I32, tag="etTi")
            nc.vector.tensor_copy(etT_i[:], etT_ps[:])
            # mask page (NP, NL): rows 0..80 = 1, rows 81+j = mask[2j+3];
            # built as (NL, NP) concat in the free axis, then PE-transposed
            mcat = cst.tile([NL, NP], F32, tag="mcat")
            nc.vector.memset(mcat[:], 1.0)
            nc.vector.tensor_copy(
                _ap(mcat, SE, [[NP, NL], [1, NJ]]),
                _ap(mfree, 1, [[SE, NL], [2, NJ]]),
            )
            mpage_ps = ps1.tile([NP, NL], F32, tag="tp")
            nc.tensor.transpose(mpage_ps[:], mcat[:], ident[:NL, :NL])
            mpage = cst.tile([NP, NL], F32, tag="mpage")
            nc.vector.tensor_copy(mpage[:], mpage_ps[:])
            # target lengths -> row (1, NL) f32
            tlf = cst.tile([NL, 1], F32, tag="tlf")
            nc.vector.tensor_copy(tlf[:], tls[:])
            tlT_ps = ps1.tile([1, NL], F32, tag="tp1")
            nc.tensor.transpose(tlT_ps[:], tlf[:], ident[:NL, :NL])
            lrow = cst.tile([1, NL], F32, tag="lrow")
            nc.vector.tensor_copy(lrow[:], tlT_ps[:])
            l2row = cst.tile([1, NL], F32, tag="l2row")
            nc.vector.tensor_scalar(l2row[:], lrow[:], 2.0, None, OP.mult)
            # thr (NP, NL) = 2L broadcast down partitions (via PE)
            thr_ps = ps1.tile([NP, NL], F32, tag="tp")
            nc.tensor.matmul(thr_ps[:], onesb[:1, :NP], l2row[:], start=True, stop=True)
            thr = cst.tile([NP, NL], F32, tag="thr")
            nc.vector.tensor_copy(thr[:], thr_ps[:])

            # per-row state value: rows 0..80 -> s, rows 81+j -> 2j+1
            siota = cst.tile([SE, 1], I32, tag="siota")
            nc.gpsimd.iota(siota[:], pattern=[[0, 1]], base=0, channel_multiplier=1)
            siof = cst.tile([SE, 1], F32, tag="siof")
            nc.vector.tensor_copy(siof[:], siota[:])
            vfree = cst.tile([1, NP], I32, tag="vfree")
            nc.gpsimd.iota(
                _ap(vfree, 0, [[NP, 1], [1, SE]]),
                pattern=[[1, SE]], base=0, channel_multiplier=0,
            )
            nc.gpsimd.iota(
                _ap(vfree, SE, [[NP, 1], [1, NJ]]),
                pattern=[[2, NJ]], base=1, channel_multiplier=0,
            )
            vfree_f = cst.tile([1, NP], F32, tag="vfreef")
            nc.vector.tensor_copy(vfree_f[:], vfree[:])
            vrow_ps = ps1.tile([NP, 1], F32, tag="tp")
            nc.tensor.transpose(vrow_ps[:], vfree_f[:], ident[:1, :1])
            vrow = cst.tile([NP, 1], F32, tag="vrow")
            nc.vector.tensor_copy(vrow[:], vrow_ps[:])
            # Wm (NP, NL): 1 iff row-state <= 2L
            wm = cst.tile([NP, NL], F32, tag="wm")
            nc.vector.tensor_tensor(
                wm[:], _ap(vrow, 0, [[1, NP], [0, NL]]), thr[:], OP.is_le
            )

            # ---------- gather offsets + indirect gather ----------
            offs = big.tile([SE, T * NL], I32, tag="offs")
            nc.gpsimd.iota(
                _ap(offs, 0, [[T * NL, SE], [NL, T], [1, NL]]),
                pattern=[[NL * C, T], [C, NL]],
                base=0,
                channel_multiplier=0,
            )
            nc.vector.tensor_tensor(
                offs[:],
                offs[:],
                _ap(etT_i, 0, [[NL, SE], [0, T], [1, NL]]),
                OP.add,
            )
            glp = big.tile([SE, T * NL], F32, tag="glp")
            nc.gpsimd.indirect_dma_start(
                glp[:],
                None,
                bass.AP(lp_ext, 0, [[C, T * NL], [1, C]]),
                bass.IndirectOffsetOnAxis(ap=offs[:], axis=1),
            )

            # ---------- P2 (NP, T*NL): per-t page [p(s); p(2j+1)*m'(2j+1)] ----------
            p2 = big.tile([NP, T * NL], F32, tag="p2")
            nc.scalar.activation(p2[:SE, :], glp[:], AF.Exp)
            nc.vector.tensor_scalar(p2[:SE, :], p2[:SE, :], E_SHIFT, None, OP.mult)
            # odd alpha rows copied down to partitions 81+j (plain-offset
            # row DMAs; strided-partition APs break dep tracking), then one
            # full-height masked multiply (rows 0..80 scale by 1.0)
            for j in range(NJ):
                nc.sync.dma_start(
                    _ap(p2, (SE + j) * T * NL, [[T * NL, 1], [1, T * NL]]),
                    _ap(p2, (2 * j + 1) * T * NL, [[T * NL, 1], [1, T * NL]]),
                )
            nc.vector.tensor_tensor(
                p2[:],
                p2[:],
                _ap(mpage, 0, [[NL, NP], [0, T], [1, NL]]),
                OP.mult,
            )

            # ---------- scan ----------
            rlog = cst.tile([1, n_ren * NL], F32, tag="rlog")

            x = xpool.tile([NP, NL], F32, tag="X")
            nc.vector.memset(x[:], 0.0)
            nc.vector.tensor_copy(x[:2, :], p2[:2, :NL])
            nc.sync.dma_start(
                _ap(x, SE * NL, [[NL, 1], [1, NL]]),
                _ap(p2, SE * T * NL, [[T * NL, 1], [1, NL]]),
            )

            jren = 0
            for t in range(1, T):
                acc = psp.tile([NP, NL], F32, tag="acc")
                nc.tensor.matmul(acc[:], w2[:], x[:], start=True, stop=True)
                xn = xpool.tile([NP, NL], F32, tag="X")
                nc.vector.tensor_tensor(
                    xn[:],
                    acc[:],
                    _ap(p2, t * NL, [[T * NL, NP], [1, NL]]),
                    OP.mult,
                )
                x = xn

                if (t % K_RENORM == 0 and t != T - 1) or t == T - 1:
                    aw = xpool.tile([NP, NL], F32, tag="X")
                    nc.vector.tensor_tensor(aw[:], x[:], wm[:], OP.mult)
                    rs = ps1.tile([1, NL], F32, tag="tp1")
                    nc.tensor.matmul(
                        rs[:], onesl[:], aw[:SE, :], start=True, stop=True
                    )
                    rr = tmp.tile([1, NL], F32, tag="rr")
                    nc.vector.reciprocal(rr[:], rs[:])
                    nc.vector.tensor_copy(
                        _ap(rlog, jren, [[n_ren * NL, 1], [n_ren, NL]]),
                        rs[:],
                    )
                    rb = ps1.tile([NP, NL], F32, tag="tp")
                    nc.tensor.matmul(
                        rb[:], onesb[:1, :NP], rr[:], start=True, stop=True
                    )
                    xs = xpool.tile([NP, NL], F32, tag="X")
                    nc.vector.tensor_tensor(xs[:], aw[:], rb[:], OP.mult)
                    xr = xpool.tile([NP, NL], F32, tag="X")
                    nc.vector.tensor_scalar(xr[:], xs[:], CLAMP, None, OP.min)
                    x = xr
                    jren += 1
            assert jren == n_ren

            # ---------- final extraction ----------
            thrm1 = tmp.tile([SE, NL], F32, tag="thrm1")
            nc.vector.tensor_scalar(thrm1[:], thr[:SE, :], 1.0, None, OP.subtract)
            ge = tmp.tile([SE, NL], F32, tag="ge")
            nc.vector.tensor_tensor(
                ge[:], _ap(siof, 0, [[1, SE], [0, NL]]), thrm1[:], OP.is_ge
            )
            le = tmp.tile([SE, NL], F32, tag="le")
            nc.vector.tensor_tensor(
                le[:], _ap(siof, 0, [[1, SE], [0, NL]]), thr[:SE, :], OP.is_le
            )
            wsel = tmp.tile([SE, NL], F32, tag="wsel")
            nc.vector.tensor_mul(wsel[:], ge[:], le[:])
            sel = tmp.tile([SE, NL], F32, tag="sel")
            nc.vector.tensor_mul(sel[:], x[:SE, :], wsel[:])
            tot = ps1.tile([1, NL], F32, tag="tp1")
            nc.tensor.matmul(tot[:], onesl[:], sel[:], start=True, stop=True)
            ltot = tmp.tile([1, NL], F32, tag="ltot")
            nc.scalar.activation(ltot[:], tot[:], AF.Ln)
            # log of scales, then sum over renorm events (n-major layout)
            lr = tmp.tile([1, n_ren * NL], F32, tag="lr")
            nc.scalar.activation(lr[:], rlog[:], AF.Ln)
            slog = tmp.tile([1, NL], F32, tag="slog")
            nc.vector.tensor_reduce(
                slog[:],
                _ap(lr, 0, [[n_ren * NL, 1], [n_ren, NL], [1, n_ren]]),
                mybir.AxisListType.X,
                OP.add,
            )
            q = tmp.tile([1, NL], F32, tag="q")
            nc.vector.tensor_add(q[:], ltot[:], slog[:])
            # (q - T*SHIFT) * -1 = T*SHIFT - q
            q2 = tmp.tile([1, NL], F32, tag="q2")
            nc.vector.tensor_scalar(q2[:], q[:], float(T) * SHIFT, -1.0, OP.subtract, OP.mult)
            rl = tmp.tile([1, NL], F32, tag="rl")
            nc.vector.reciprocal(rl[:], lrow[:])
            loss = tmp.tile([1, NL], F32, tag="loss")
            nc.vector.tensor_mul(loss[:], q2[:], rl[:])
            nc.sync.dma_start(out_ext[:], loss[:])

    nc.compile()
    return nc


_NC_CACHE = {}


def _get_nc(T=T_FULL):
    if T not in _NC_CACHE:
        _NC_CACHE[T] = build_nc(T)
    return _NC_CACHE[T]


def kernel(log_probs, targets, input_lengths, target_lengths):
    lp = np.ascontiguousarray(np.asarray(log_probs, dtype=np.float32))
    tg = np.ascontiguousarray(np.asarray(targets, dtype=np.int32))
    tl = np.ascontiguousarray(np.asarray(target_lengths, dtype=np.int32))
    T, N, _ = lp.shape
    nc = _get_nc(T)
    in_maps = []
    for i in range(NC_CORES):
        s = slice(i * NL, (i + 1) * NL)
        in_maps.append(
            {
                "log_probs": np.ascontiguousarray(lp[:, s, :]),
                "targets": np.ascontiguousarray(tg[s]),
                "target_lengths": np.ascontiguousarray(tl[s]),
            }
        )
    res = run_bass_kernel_spmd(nc, in_maps, core_ids=list(range(NC_CORES)))
    out = np.concatenate([res.results[i]["out"].reshape(NL) for i in range(NC_CORES)])
    return out.astype(np.float32)


# revision 8
# speedup vs baseline: 2.0853x; 1.8658x over previous
"""CTC loss forward on 8 TRN2 NeuronCores, data-parallel over batch.

Problem: log_probs (512, 32, 8000) f32, targets (32, 40) i32,
target_lengths (32,) i32 -> per-sample loss (32,) f32
(input_lengths is ignored, matching the reference).

Strategy per core (4 samples):
 - Gather only the needed log-prob entries: glp[s, t, n] = lp[t, n, et[n, s]]
   (T*4*81 = 166K elements) via one indirect DMA; the 512MB tensor is
   never streamed.
 - Run the T-step DP in linear probability space with an augmented state
   on partitions: rows 0..80 = alpha over the 81 CTC states, rows
   81..119 = the 39 masked skip terms am[j] = alpha[2j+1]*mask[2j+3].
   One constant 120x120 matmul performs all shifts AND regenerates the
   duplicated skip rows; one FD=4 DVE multiply by the precomputed
   per-step probability page completes the step:
       X' = (W2 @ X) * P2[:, t]
 - Every K=8 steps renormalize by the per-sample sum of alpha over
   states s <= 2L (window excludes padding states that run away),
   clamp, and log-accumulate the scales.
 - loss = -(log(alpha[2L] + alpha[2L-1]) + sum(log scales) - T*SHIFT)/L
"""
import sys

for _p in ("/opt/trn_rl_repo",):
    if _p not in sys.path:
        sys.path.append(_p)

import numpy as np
import concourse.bass as bass
import concourse.bacc as bacc
import concourse.mybir as mybir
from concourse import tile
from concourse.bass_utils import run_bass_kernel_spmd

F32 = mybir.dt.float32
I32 = mybir.dt.int32
BF = mybir.dt.bfloat16
AF = mybir.ActivationFunctionType
OP = mybir.AluOpType

T_FULL = 512
NL = 4          # samples per core
NC_CORES = 8
C = 8000
S = 40
SE = 2 * S + 1  # 81
NJ = 39         # skip rows: odd states 1,3,..,77
NP = SE + NJ    # 120 partitions of augmented state
K_RENORM = 16
SHIFT = 9.0
E_SHIFT = float(np.float32(np.exp(np.float32(SHIFT))))
CLAMP = 1e26


def _ap(t, off, dims):
    a = t[:]
    return bass.AP(a.tensor, off, [list(d) for d in dims])


def build_nc(T=T_FULL):
    nc = bacc.Bacc("TRN2", target_bir_lowering=False, debug=True)
    lp_ext = nc.declare_dram_parameter("log_probs", [T, NL, C], F32, isOutput=False)
    tg_ext = nc.declare_dram_parameter("targets", [NL, S], I32, isOutput=False)
    tl_ext = nc.declare_dram_parameter("target_lengths", [NL], I32, isOutput=False)
    out_ext = nc.declare_dram_parameter("out", [1, NL], F32, isOutput=True)

    n_ren = len([t for t in range(1, T) if t % K_RENORM == 0 and t != T - 1]) + 1

    with tile.TileContext(nc) as tc:
        with (
            tc.tile_pool(name="cst", bufs=1) as cst,
            tc.tile_pool(name="big", bufs=1) as big,
            tc.tile_pool(name="x", bufs=3) as xpool,
            tc.tile_pool(name="tmp", bufs=2) as tmp,
            tc.tile_pool(name="ps", bufs=2, space=bass.MemorySpace.PSUM) as psp,
            tc.tile_pool(name="ps1", bufs=2, space=bass.MemorySpace.PSUM) as ps1,
        ):
            # ---------- constants built on device ----------
            dmat = cst.tile([128, 128], I32, tag="dmat")
            nc.gpsimd.iota(dmat[:], pattern=[[1, 128]], base=0, channel_multiplier=-1)
            ident = cst.tile([128, 128], F32, tag="ident")
            nc.vector.tensor_scalar(ident[:], dmat[:], 0, None, OP.is_equal)

            # W2 lhsT (NP, NP): lhsT[c, o] = W2[o, c]
            w2 = cst.tile([NP, NP], BF, tag="w2")
            nc.vector.memset(w2[:], 0.0)
            # [0:81, 0:81]: 1 iff o - c in {0, 1}
            ge0 = tmp.tile([SE, SE], F32, tag="scr0")
            nc.vector.tensor_scalar(ge0[:], dmat[:SE, :SE], 0, None, OP.is_ge)
            le1 = tmp.tile([SE, SE], F32, tag="scr1")
            nc.vector.tensor_scalar(le1[:], dmat[:SE, :SE], 1, None, OP.is_le)
            nc.vector.tensor_mul(_ap(w2, 0, [[NP, SE], [1, SE]]), ge0[:], le1[:])
            # [0:81, 81:120]: lhsT[c, 81+j] = 1 iff c - 2j in {0, 1}
            i2 = cst.tile([SE, NJ], I32, tag="i2")
            nc.gpsimd.iota(i2[:], pattern=[[-2, NJ]], base=0, channel_multiplier=1)
            gA = tmp.tile([SE, NJ], F32, tag="gA")
            nc.vector.tensor_scalar(gA[:], i2[:], 0, None, OP.is_ge)
            gB = tmp.tile([SE, NJ], F32, tag="gB")
            nc.vector.tensor_scalar(gB[:], i2[:], 1, None, OP.is_le)
            nc.vector.tensor_mul(_ap(w2, SE, [[NP, SE], [1, NJ]]), gA[:], gB[:])
            # rows 81:120 built at base partitions, then DMA'd into place
            scrI = cst.tile([NJ, NP], I32, tag="scrI")
            # cols 0:81: 1 iff f - 2j - 3 == 0
            nc.gpsimd.iota(
                _ap(scrI, 0, [[NP, NJ], [1, SE]]),
                pattern=[[1, SE]], base=-3, channel_multiplier=-2,
            )
            # cols 81:120: 1 iff f - j - 1 == 0
            nc.gpsimd.iota(
                _ap(scrI, SE, [[NP, NJ], [1, NJ]]),
                pattern=[[1, NJ]], base=-1, channel_multiplier=-1,
            )
            scrF = cst.tile([NJ, NP], BF, tag="scrF")
            nc.vector.tensor_scalar(scrF[:], scrI[:], 0, None, OP.is_equal)
            nc.sync.dma_start(_ap(w2, SE * NP, [[NP, NJ], [1, NP]]), scrF[:])

            onesl = cst.tile([SE, 1], BF, tag="onesl")
            nc.vector.memset(onesl[:], 1.0)
            onesb = cst.tile([1, NP], BF, tag="onesb")
            nc.vector.memset(onesb[:], 1.0)
            onesbf = cst.tile([1, NP], F32, tag="onesbf")
            nc.vector.memset(onesbf[:], 1.0)

            # ---------- small inputs ----------
            tgs = cst.tile([NL, S], I32, tag="tgs")
            nc.sync.dma_start(tgs[:], tg_ext[:])
            tls = cst.tile([NL, 1], I32, tag="tls")
            nc.sync.dma_start(tls[:], _ap(tl_ext, 0, [[1, NL], [1, 1]]))

            # et (NL, SE) f32: blank-expanded targets; odd slots get labels
            et = cst.tile([NL, SE], F32, tag="et")
            nc.vector.memset(et[:], 0.0)
            nc.vector.tensor_copy(_ap(et, 1, [[SE, NL], [2, S]]), tgs[:])
            # mfree (NL, SE): col s' holds mask at dest s'+2 = (et[s'+2] != et[s'])
            mfree = cst.tile([NL, SE], F32, tag="mfree")
            nc.vector.memset(mfree[:], 0.0)
            nc.vector.tensor_tensor(
                _ap(mfree, 0, [[SE, NL], [1, SE - 2]]),
                _ap(et, 2, [[SE, NL], [1, SE - 2]]),
                _ap(et, 0, [[SE, NL], [1, SE - 2]]),
                OP.not_equal,
            )

            # ---------- transposes to states-on-partitions ----------
            etT_ps = ps1.tile([SE, NL], F32, tag="tp")
            nc.tensor.transpose(etT_ps[:], et[:], ident[:NL, :NL])
            etT_i = cst.tile([SE, NL], I32, tag="etTi")
            nc.vector.tensor_copy(etT_i[:], etT_ps[:])
            # mask page (NP, NL): rows 0..80 = 1, rows 81+j = mask[2j+3];
            # built as (NL, NP) concat in the free axis, then PE-transposed
            mcat = cst.tile([NL, NP], F32, tag="mcat")
            nc.vector.memset(mcat[:], 1.0)
            nc.vector.tensor_copy(
                _ap(mcat, SE, [[NP, NL], [1, NJ]]),
                _ap(mfree, 1, [[SE, NL], [2, NJ]]),
            )
            mpage_ps = ps1.tile([NP, NL], F32, tag="tp")
            nc.tensor.transpose(mpage_ps[:], mcat[:], ident[:NL, :NL])
            mpage = cst.tile([NP, NL], BF, tag="mpage")
            nc.vector.tensor_copy(mpage[:], mpage_ps[:])
            # target lengths -> row (1, NL) f32
            tlf = cst.tile([NL, 1], F32, tag="tlf")
            nc.vector.tensor_copy(tlf[:], tls[:])
            tlT_ps = ps1.tile([1, NL], F32, tag="tp1")
            nc.tensor.transpose(tlT_ps[:], tlf[:], ident[:NL, :NL])
            lrow = cst.tile([1, NL], F32, tag="lrow")
            nc.vector.tensor_copy(lrow[:], tlT_ps[:])
            l2row = cst.tile([1, NL], F32, tag="l2row")
            nc.vector.tensor_scalar(l2row[:], lrow[:], 2.0, None, OP.mult)
            # thr (NP, NL) = 2L broadcast down partitions (via PE)
            thr_ps = ps1.tile([NP, NL], F32, tag="tp")
            nc.tensor.matmul(thr_ps[:], onesbf[:1, :NP], l2row[:], start=True, stop=True)
            thr = cst.tile([NP, NL], F32, tag="thr")
            nc.vector.tensor_copy(thr[:], thr_ps[:])

            # per-row state value: rows 0..80 -> s, rows 81+j -> 2j+1
            siota = cst.tile([SE, 1], I32, tag="siota")
            nc.gpsimd.iota(siota[:], pattern=[[0, 1]], base=0, channel_multiplier=1)
            siof = cst.tile([SE, 1], F32, tag="siof")
            nc.vector.tensor_copy(siof[:], siota[:])
            vfree = cst.tile([1, NP], I32, tag="vfree")
            nc.gpsimd.iota(
                _ap(vfree, 0, [[NP, 1], [1, SE]]),
                pattern=[[1, SE]], base=0, channel_multiplier=0,
            )
            nc.gpsimd.iota(
                _ap(vfree, SE, [[NP, 1], [1, NJ]]),
                pattern=[[2, NJ]], base=1, channel_multiplier=0,
            )
            vfree_f = cst.tile([1, NP], F32, tag="vfreef")
            nc.vector.tensor_copy(vfree_f[:], vfree[:])
            vrow_ps = ps1.tile([NP, 1], F32, tag="tp")
            nc.tensor.transpose(vrow_ps[:], vfree_f[:], ident[:1, :1])
            vrow = cst.tile([NP, 1], F32, tag="vrow")
            nc.vector.tensor_copy(vrow[:], vrow_ps[:])
            # Wm (NP, NL): 1 iff row-state <= 2L
            wm = cst.tile([NP, NL], BF, tag="wm")
            nc.vector.tensor_tensor(
                wm[:], _ap(vrow, 0, [[1, NP], [0, NL]]), thr[:], OP.is_le
            )

            # ---------- gather offsets + indirect gather ----------
            offs = big.tile([SE, T * NL], I32, tag="offs")
            nc.gpsimd.iota(
                _ap(offs, 0, [[T * NL, SE], [NL, T], [1, NL]]),
                pattern=[[NL * C, T], [C, NL]],
                base=0,
                channel_multiplier=0,
            )
            nc.vector.tensor_tensor(
                offs[:],
                offs[:],
                _ap(etT_i, 0, [[NL, SE], [0, T], [1, NL]]),
                OP.add,
            )
            glp = big.tile([SE, T * NL], F32, tag="glp")
            nc.gpsimd.indirect_dma_start(
                glp[:],
                None,
                bass.AP(lp_ext, 0, [[C, T * NL], [1, C]]),
                bass.IndirectOffsetOnAxis(ap=offs[:], axis=1),
            )

            # ---------- P2 (NP, T*NL): per-t page [p(s); p(2j+1)*m'(2j+1)] ----------
            p2 = big.tile([NP, T * NL], BF, tag="p2")
            nc.scalar.activation(p2[:SE, :], glp[:], AF.Exp)
            nc.vector.tensor_scalar(p2[:SE, :], p2[:SE, :], E_SHIFT, None, OP.mult)
            # odd alpha rows copied down to partitions 81+j (plain-offset
            # row DMAs; strided-partition APs break dep tracking), then one
            # full-height masked multiply (rows 0..80 scale by 1.0)
            for j in range(NJ):
                nc.sync.dma_start(
                    _ap(p2, (SE + j) * T * NL, [[T * NL, 1], [1, T * NL]]),
                    _ap(p2, (2 * j + 1) * T * NL, [[T * NL, 1], [1, T * NL]]),
                )
            nc.vector.tensor_tensor(
                p2[:],
                p2[:],
                _ap(mpage, 0, [[NL, NP], [0, T], [1, NL]]),
                OP.mult,
            )

            # ---------- scan ----------
            rlog = cst.tile([1, n_ren * NL], F32, tag="rlog")

            x = xpool.tile([NP, NL], BF, tag="X")
            nc.vector.memset(x[:], 0.0)
            nc.vector.tensor_copy(x[:2, :], p2[:2, :NL])
            nc.sync.dma_start(
                _ap(x, SE * NL, [[NL, 1], [1, NL]]),
                _ap(p2, SE * T * NL, [[T * NL, 1], [1, NL]]),
            )

            jren = 0
            for t in range(1, T):
                acc = psp.tile([NP, NL], F32, tag="acc")
                nc.tensor.matmul(acc[:], w2[:], x[:], start=True, stop=True)
                xn = xpool.tile([NP, NL], BF, tag="X")
                nc.vector.tensor_tensor(
                    xn[:],
                    acc[:],
                    _ap(p2, t * NL, [[T * NL, NP], [1, NL]]),
                    OP.mult,
                )
                x = xn

                if (t % K_RENORM == 0 and t != T - 1) or t == T - 1:
                    aw = xpool.tile([NP, NL], BF, tag="X")
                    nc.vector.tensor_tensor(aw[:], x[:], wm[:], OP.mult)
                    rs = ps1.tile([1, NL], F32, tag="tp1")
                    nc.tensor.matmul(
                        rs[:], onesl[:], aw[:SE, :], start=True, stop=True
                    )
                    rr = tmp.tile([1, NL], BF, tag="rr")
                    with nc.allow_low_precision(reason="renorm scale; log uses exact rs"):
                        nc.vector.reciprocal(rr[:], rs[:])
                    nc.vector.tensor_copy(
                        _ap(rlog, jren, [[n_ren * NL, 1], [n_ren, NL]]),
                        rs[:],
                    )
                    rb = ps1.tile([NP, NL], F32, tag="tp")
                    nc.tensor.matmul(
                        rb[:], onesb[:1, :NP], rr[:], start=True, stop=True
                    )
                    xs = xpool.tile([NP, NL], BF, tag="X")
                    nc.vector.tensor_tensor(xs[:], aw[:], rb[:], OP.mult)
                    xr = xpool.tile([NP, NL], BF, tag="X")
                    nc.vector.tensor_scalar(xr[:], xs[:], CLAMP, None, OP.min)
                    x = xr
                    jren += 1
            assert jren == n_ren

            # ---------- final extraction ----------
            thrm1 = tmp.tile([SE, NL], F32, tag="thrm1")
            nc.vector.tensor_scalar(thrm1[:], thr[:SE, :], 1.0, None, OP.subtract)
            ge = tmp.tile([SE, NL], F32, tag="ge")
            nc.vector.tensor_tensor(
                ge[:], _ap(siof, 0, [[1, SE], [0, NL]]), thrm1[:], OP.is_ge
            )
            le = tmp.tile([SE, NL], F32, tag="le")
            nc.vector.tensor_tensor(
                le[:], _ap(siof, 0, [[1, SE], [0, NL]]), thr[:SE, :], OP.is_le
            )
            wsel = tmp.tile([SE, NL], BF, tag="wsel")
            nc.vector.tensor_mul(wsel[:], ge[:], le[:])
            sel = tmp.tile([SE, NL], BF, tag="sel")
            nc.vector.tensor_mul(sel[:], x[:SE, :], wsel[:])
            tot = ps1.tile([1, NL], F32, tag="tp1")
            nc.tensor.matmul(tot[:], onesl[:], sel[:], start=True, stop=True)
            ltot = tmp.tile([1, NL], F32, tag="ltot")
            nc.scalar.activation(ltot[:], tot[:], AF.Ln)
            # log of scales, then sum over renorm events (n-major layout)
            lr = tmp.tile([1, n_ren * NL], F32, tag="lr")
            nc.scalar.activation(lr[:], rlog[:], AF.Ln)
            slog = tmp.tile([1, NL], F32, tag="slog")
            nc.vector.tensor_reduce(
                slog[:],
                _ap(lr, 0, [[n_ren * NL, 1], [n_ren, NL], [1, n_ren]]),
                mybir.AxisListType.X,
                OP.add,
            )
            q = tmp.tile([1, NL], F32, tag="q")
            nc.vector.tensor_add(q[:], ltot[:], slog[:])
            # (q - T*SHIFT) * -1 = T*SHIFT - q
            q2 = tmp.tile([1, NL], F32, tag="q2")
            nc.vector.tensor_scalar(q2[:], q[:], float(T) * SHIFT, -1.0, OP.subtract, OP.mult)
            rl = tmp.tile([1, NL], F32, tag="rl")
            nc.vector.reciprocal(rl[:], lrow[:])
            loss = tmp.tile([1, NL], F32, tag="loss")
            nc.vector.tensor_mul(loss[:], q2[:], rl[:])
            nc.sync.dma_start(out_ext[:], loss[:])

    nc.compile()
    return nc


_NC_CACHE = {}


def _get_nc(T=T_FULL):
    if T not in _NC_CACHE:
        _NC_CACHE[T] = build_nc(T)
    return _NC_CACHE[T]


def kernel(log_probs, targets, input_lengths, target_lengths):
    lp = np.ascontiguousarray(np.asarray(log_probs, dtype=np.float32))
    tg = np.ascontiguousarray(np.asarray(targets, dtype=np.int32))
    tl = np.ascontiguousarray(np.asarray(target_lengths, dtype=np.int32))
    T, N, _ = lp.shape
    nc = _get_nc(T)
    in_maps = []
    for i in range(NC_CORES):
        s = slice(i * NL, (i + 1) * NL)
        in_maps.append(
            {
                "log_probs": np.ascontiguousarray(lp[:, s, :]),
                "targets": np.ascontiguousarray(tg[s]),
                "target_lengths": np.ascontiguousarray(tl[s]),
            }
        )
    res = run_bass_kernel_spmd(nc, in_maps, core_ids=list(range(NC_CORES)))
    out = np.concatenate([res.results[i]["out"].reshape(NL) for i in range(NC_CORES)])
    return out.astype(np.float32)
